# revision 17
# baseline (speedup 1.0000x reference)
"""Trainium2 Bass kernel for one pre-LN transformer block (B=2, T=2048, D=768,
H=12 causal attention + 4x MLP), sharded over 8 NeuronCores.

Sharding (SPMD, one NEFF for all cores):
  * 2 batch groups x 4 cores.  Within a group, attention is tensor-parallel
    over heads (3 heads/core, full 2048-token causal attention), producing a
    partial out-projection y_c.  One chunked ReduceScatter(add) per 512-token
    q-block both sums the head contributions and token-shards the result.
  * The MLP sublayer is token-parallel and pipelined: each core runs LN2 +
    MLP on each 128-token shard tile as soon as its ReduceScatter chunk
    lands, overlapping the remaining attention blocks.
  * LayerNorm gains/biases are folded into the weights host-side.  rstd is
    computed with a table-free Newton iteration (reciprocal + polish), so the
    scalar engine needs exactly one activation-table set (exp/relu/copy).

Matmul structure:
  * LN1 output is transposed 128x128-wise on the PE; all 6 transposes of a
    tile share one PSUM bank (start=True only on the first) and leave in a
    single wide copy.
  * Attention per q-block b (512 queries), per head: for each k-tile one
    wide scores matmul (K stationary), one exp() ACT, and one V-stationary
    matmul accumulating O^T[65, 512] (extra ones-column gives the softmax
    denominator).  The diagonal k-tile is truncated to the causal width and
    its first q-tile masked on the vector engine.
  * O^T is divided by the denominator and written into a merged [128, 2, 512]
    lhsT tile (head 1 lands partition-shifted to rows 64-127), so the out
    projection is 2 matmuls per 512-col output slice.
  * MLP up-projection is computed token-tile-wise with the weight as the
    moving operand (rhs width 512 regardless of token batching), relu on
    gpsimd, transposed on the PE, down-projection with aT stationary.

All matmuls run in bf16 (weights pre-cast on host) with fp32 PSUM
accumulation; layernorm, softmax normalization and residuals are fp32.
"""

import math
from contextlib import ExitStack

import ml_dtypes
import numpy as np

import concourse.bass as bass
import concourse.bacc as bacc_mod
import concourse.mybir as mybir
import concourse.tile as tile
from concourse.bass import ds
from concourse.bass_utils import run_bass_kernel_spmd
from concourse.masks import make_identity

B, T, D, H, HD = 2, 2048, 768, 12, 64
DH = 4 * D                  # 3072 mlp hidden
EPS = 1e-5
NCORES = 8
GRP = 4                     # cores per batch group
HPC = H // GRP              # 3 heads per core
DLOC = HPC * HD             # 192 local head dims
DPAD = 256                  # local head dims padded to 2x128
TSH = T // GRP              # 512-token shard for the MLP phase
NT = T // 128               # 16 token tiles per batch
NB = 4                      # q-blocks (512 queries each)
NSH = TSH // 128            # 4 shard tiles
KD = D // 128               # 6
KH = DH // 128              # 24
SCALE = HD ** -0.5

BF = mybir.dt.bfloat16
F32 = mybir.dt.float32
AF = mybir.ActivationFunctionType
ALU = mybir.AluOpType
BF_NP = ml_dtypes.bfloat16


def _newton_rstd(nc, pool, var_ap, rstd_out, n):
    """rstd_out[128, n] = (var + EPS) ** -0.5, table-free.

    y0 = 1/(v+eps) from the DVE reciprocal, then 4 Newton steps
    y <- y * (1.5 - 0.5 * (v+eps) * y^2).  Converges to <1e-6 rel for
    v in [0.2, 3]; our rows have v ~= 1."""
    t = pool.tile([128, n], F32, tag="nw_t", name="nw_t", bufs=2)
    u = pool.tile([128, n], F32, tag="nw_u", name="nw_u", bufs=2)
    nc.vector.tensor_scalar(out=t, in0=var_ap, scalar1=EPS, scalar2=None,
                            op0=ALU.add)
    nc.vector.reciprocal(out=rstd_out, in_=t)
    for _ in range(4):
        nc.vector.tensor_tensor(out=u, in0=rstd_out, in1=rstd_out, op=ALU.mult)
        nc.vector.tensor_tensor(out=u, in0=u, in1=t, op=ALU.mult)
        nc.vector.tensor_scalar(out=u, in0=u, scalar1=-0.5, scalar2=1.5,
                                op0=ALU.mult, op1=ALU.add)
        nc.vector.tensor_tensor(out=rstd_out, in0=rstd_out, in1=u, op=ALU.mult)


def build_nc():
    nc = bacc_mod.Bacc(None, num_devices=NCORES)

    # ---- per-core external I/O (host does the slicing / padding) ----
    x_full = nc.dram_tensor("x_full", [NT, 128, D], BF, kind="ExternalInput")
    x_own = nc.dram_tensor("x_own", [NSH, 128, D], F32, kind="ExternalInput")
    wq_s = nc.dram_tensor("wq_s", [KD, 128, DLOC], BF, kind="ExternalInput")
    wk_s = nc.dram_tensor("wk_s", [KD, 128, DLOC], BF, kind="ExternalInput")
    wv_s = nc.dram_tensor("wv_s", [KD, 128, DLOC], BF, kind="ExternalInput")
    wo_s = nc.dram_tensor("wo_s", [2, 128, D], BF, kind="ExternalInput")
    w1_e = nc.dram_tensor("w1_e", [KD, 128, DH], BF, kind="ExternalInput")
    w2_e = nc.dram_tensor("w2_e", [KH, 128, D], BF, kind="ExternalInput")
    bq_s = nc.dram_tensor("bq_s", [2, 128], F32, kind="ExternalInput")
    bk_s = nc.dram_tensor("bk_s", [2, 128], F32, kind="ExternalInput")
    bv_s = nc.dram_tensor("bv_s", [DLOC], F32, kind="ExternalInput")
    b1_s = nc.dram_tensor("b1_s", [DH], BF, kind="ExternalInput")
    y_out = nc.dram_tensor("y_out", [NSH, 128, D], F32, kind="ExternalOutput")

    with tile.TileContext(nc) as tc, ExitStack() as ctx:
        # ---------------- pools ----------------
        consts = ctx.enter_context(tc.tile_pool(name="consts", bufs=1))
        state = ctx.enter_context(tc.tile_pool(name="state", bufs=1))
        dram = ctx.enter_context(
            tc.tile_pool(name="dram", bufs=1, space="DRAM"))
        stats = ctx.enter_context(tc.tile_pool(name="stats", bufs=4))
        work = ctx.enter_context(tc.tile_pool(name="work", bufs=3))
        exps = ctx.enter_context(tc.tile_pool(name="exps", bufs=3))
        outw = ctx.enter_context(tc.tile_pool(name="outw", bufs=2))
        # psum: mm 3 banks + scores 2x1 + O 2x1 = 7 banks (1 spare)
        pmm = ctx.enter_context(tc.tile_pool(name="pmm", bufs=3, space="PSUM"))
        psc = ctx.enter_context(tc.tile_pool(name="psc", bufs=2, space="PSUM"))
        po = ctx.enter_context(tc.tile_pool(name="po", bufs=2, space="PSUM"))

        # ---------------- constants ----------------
        ident = consts.tile([128, 128], BF, tag="ident")
        make_identity(nc, ident)
        # causal mask for diagonal tiles: cmask[k, q] = 1.0 iff q >= k
        cmask = consts.tile([128, 128], BF, tag="cmask")
        nc.vector.memset(cmask, 1.0)
        nc.gpsimd.affine_select(
            out=cmask, in_=cmask, compare_op=ALU.is_ge, fill=0.0, base=0,
            pattern=[[1, 128]], channel_multiplier=-1)
        ones_r = consts.tile([1, 64], BF, tag="ones_r")
        nc.vector.memset(ones_r, 1.0)
        bq_sb = consts.tile([128, 2], F32, tag="bq")
        bk_sb = consts.tile([128, 2], F32, tag="bk")
        for ko in range(2):
            nc.sync.dma_start(out=bq_sb[:, ko:ko + 1], in_=bq_s[ko, :, None])
            nc.sync.dma_start(out=bk_sb[:, ko:ko + 1], in_=bk_s[ko, :, None])
        bv_rep = consts.tile([128, DLOC], F32, tag="bv")
        bv_ap = bv_s[:]
        nc.sync.dma_start(
            out=bv_rep,
            in_=bass.AP(tensor=bv_ap.tensor, offset=bv_ap.offset,
                        ap=[[0, 128]] + list(bv_ap.ap)))
        b1_rep = consts.tile([128, DH], BF, tag="b1r")
        b1_ap = b1_s[:]
        nc.sync.dma_start(
            out=b1_rep,
            in_=bass.AP(tensor=b1_ap.tensor, offset=b1_ap.offset,
                        ap=[[0, 128]] + list(b1_ap.ap)))

        # ---------------- persistent state ----------------
        # Q/K with head i at (tile 64i//128, base 64i%128); tile1 rows 64-127
        # unused.
        QT = state.tile([128, 2, T], BF, tag="QT")
        KT = state.tile([128, 2, T], BF, tag="KT")
        Vx = state.tile([128, NT, HPC, 65], BF, tag="Vx")  # V + ones col
        hT = state.tile([128, KD, T], BF, tag="hT")
        xr = state.tile([128, NSH, D], BF, tag="xr")  # x + attn residual
        # weights (bf16, host-cast)
        wq_sb = state.tile([128, KD, DLOC], BF, tag="wq")
        wk_sb = state.tile([128, KD, DLOC], BF, tag="wk")
        wv_sb = state.tile([128, KD, DLOC], BF, tag="wv")
        wo_sb = state.tile([128, 2, D], BF, tag="wo")
        w1_sb = state.tile([128, KD, DH], BF, tag="w1")
        w2_sb = state.tile([128, KH, D], BF, tag="w2")
        mv1 = consts.tile([128, NT, 2], F32, tag="mv1")
        rstd1 = consts.tile([128, NT], F32, tag="rstd1")
        mv2 = consts.tile([128, NSH, 2], F32, tag="mv2")
        rstd2 = consts.tile([128, NSH], F32, tag="rstd2")
        # ones columns of extended-V (softmax denominators for free)
        nc.vector.memset(Vx[:, :, :, 64:65], 1.0)

        # collective buffers
        cc_in = dram.tile([NT, 128, D], BF, tag="cc_in")
        cc_rs = dram.tile([NSH, 128, D], BF, tag="cc_rs")

        def head_qk(t_sb, i, col, n):
            kd_i, base = (64 * i) // 128, (64 * i) % 128
            return t_sb[base:base + 64, kd_i, ds(col, n)]

        def ln_tile(x_t, mv_out):
            st = stats.tile([128, 3, 6], F32, tag="st", name="ln_st")
            for s in range(3):
                nc.vector.bn_stats(out=st[:, s, :], in_=x_t[:, ds(256 * s, 256)])
            nc.vector.bn_aggr(out=mv_out, in_=st)

        def transpose6(src_bf, dst_ap, on_scalar):
            """6 PE transposes of a [128, 768] bf16 tile through one PSUM
            bank, then one wide copy to dst_ap ([128, 6, 128] view)."""
            ps_tr = pmm.tile([128, KD, 128], BF, tag="mm", name="ps_tr")
            for kd in range(KD):
                nc.tensor.matmul(
                    ps_tr[:, kd, :], lhsT=src_bf[:, ds(128 * kd, 128)],
                    rhs=ident, is_transpose=True,
                    start=(kd == 0), stop=True,
                    skip_group_check=(kd != 0))
            if on_scalar:
                nc.scalar.copy(out=dst_ap, in_=ps_tr)
            else:
                nc.vector.tensor_copy(out=dst_ap, in_=ps_tr)

        def mlp_tile(tt):
            """Residual + LN2 + MLP for shard tile tt (after RS chunk tt)."""
            y_sb = work.tile([128, D], BF, tag="y_sb", name="y_sb", bufs=2)
            nc.sync.dma_start(out=y_sb, in_=cc_rs[tt])
            xo = work.tile([128, D], F32, tag="xo", name="xo", bufs=2)
            nc.sync.dma_start(out=xo, in_=x_own[tt])
            nc.gpsimd.tensor_tensor(
                out=xr[:, tt, :], in0=xo, in1=y_sb, op=ALU.add)
            ln_tile(xr[:, tt, :], mv2[:, tt, :])
            _newton_rstd(nc, stats, mv2[:, tt, 1:2], rstd2[:, tt:tt + 1], 1)
            h2 = work.tile([128, D], BF, tag="h_bf", name="h2", bufs=2)
            nc.vector.tensor_scalar(
                out=h2, in0=xr[:, tt, :], scalar1=mv2[:, tt, 0:1],
                scalar2=rstd2[:, tt:tt + 1], op0=ALU.subtract, op1=ALU.mult)
            h2t = work.tile([128, KD, 128], BF, tag="h2t", name="h2t", bufs=2)
            transpose6(h2, h2t, True)
            # up-projection: weight is the moving operand (512-wide rhs)
            aT_t = work.tile([128, KH, 128], BF, tag="aT_t", name="aT_t",
                             bufs=2)
            for sl in range(KH // 4):
                ps_up = pmm.tile([128, 512], F32, tag="mm", name="ps_up")
                for kd in range(KD):
                    nc.tensor.matmul(
                        ps_up,
                        lhsT=h2t[:, kd, :],
                        rhs=w1_sb[:, kd, ds(512 * sl, 512)],
                        start=(kd == 0), stop=(kd == KD - 1),
                    )
                a_bf = work.tile([128, 512], BF, tag="a_bf", name="a_bf",
                                 bufs=2)
                nc.vector.tensor_tensor(
                    out=a_bf, in0=ps_up, in1=b1_rep[:, ds(512 * sl, 512)],
                    op=ALU.add)
                nc.gpsimd.tensor_relu(out=a_bf, in_=a_bf)
                # transpose the 4 kh-tiles of this slice through one bank
                ps_at = pmm.tile([128, 4, 128], BF, tag="mm", name="ps_at")
                for k4 in range(4):
                    nc.tensor.matmul(
                        ps_at[:, k4, :], lhsT=a_bf[:, ds(128 * k4, 128)],
                        rhs=ident, is_transpose=True,
                        start=(k4 == 0), stop=True,
                        skip_group_check=(k4 != 0))
                nc.vector.tensor_copy(out=aT_t[:, ds(4 * sl, 4), :], in_=ps_at)
            # down-projection + final residual
            out_t = work.tile([128, D], F32, tag="out_t", name="out_t",
                              bufs=2)
            for nsl, nsz in ((0, 512), (512, 256)):
                ps_dn = pmm.tile([128, 512], F32, tag="mm", name="ps_dn")
                for dh in range(KH):
                    nc.tensor.matmul(
                        ps_dn[:, 0:nsz],
                        lhsT=aT_t[:, dh, :],
                        rhs=w2_sb[:, dh, ds(nsl, nsz)],
                        start=(dh == 0), stop=(dh == KH - 1),
                    )
                nc.vector.tensor_tensor(
                    out=out_t[:, ds(nsl, nsz)], in0=ps_dn[:, 0:nsz],
                    in1=xr[:, tt, ds(nsl, nsz)], op=ALU.add)
            nc.sync.dma_start(out=y_out[tt], in_=out_t)

        # ================= main pipeline =================
        for c in range(NB):
            # ---- LN1 chunk c: dma -> stats -> rstd -> normalize+transpose
            x_ts = []
            for j, gt in enumerate(range(4 * c, 4 * c + 4)):
                x_t = work.tile([128, D], BF, tag="x_t", name="x_t", bufs=4)
                nc.sync.dma_start(out=x_t, in_=x_full[gt])
                x_ts.append(x_t)
            if c == 0:
                for kd in range(KD):
                    nc.sync.dma_start(out=wk_sb[:, kd, :], in_=wk_s[kd])
                    nc.sync.dma_start(out=wq_sb[:, kd, :], in_=wq_s[kd])
                    nc.sync.dma_start(out=wv_sb[:, kd, :], in_=wv_s[kd])
            for j, gt in enumerate(range(4 * c, 4 * c + 4)):
                ln_tile(x_ts[j], mv1[:, gt, :])
            _newton_rstd(nc, stats, mv1[:, ds(4 * c, 4), 1],
                         rstd1[:, ds(4 * c, 4)], 4)
            for j, gt in enumerate(range(4 * c, 4 * c + 4)):
                h_bf = work.tile([128, D], BF, tag="h_bf", name="h_bf",
                                 bufs=2)
                nc.vector.tensor_scalar(
                    out=h_bf, in0=x_ts[j], scalar1=mv1[:, gt, 0:1],
                    scalar2=rstd1[:, gt:gt + 1], op0=ALU.subtract, op1=ALU.mult)
                transpose6(h_bf, hT[:, :, ds(128 * gt, 128)], gt % 2 == 0)

            # ---- QKV chunk c ----
            csl = ds(512 * c, 512)
            for w_sb, t_sb, b_sb in ((wk_sb, KT, bk_sb), (wq_sb, QT, bq_sb)):
                ps = pmm.tile([128, 512], F32, tag="mm", name="ps_qk")
                for kd in range(KD):
                    nc.tensor.matmul(
                        ps, lhsT=w_sb[:, kd, 0:128], rhs=hT[:, kd, csl],
                        start=(kd == 0), stop=(kd == KD - 1),
                    )
                nc.vector.tensor_scalar(
                    out=t_sb[:, 0, csl], in0=ps, scalar1=b_sb[:, 0:1],
                    scalar2=None, op0=ALU.add)
            # head-2 Q and K col-tiled into concurrent halves of the PE
            ps_c = pmm.tile([128, 512], F32, tag="mm", name="ps_c")
            ps_d = pmm.tile([128, 512], F32, tag="mm", name="ps_d")
            for kd in range(KD):
                nc.tensor.matmul(
                    ps_c[0:64, :], lhsT=wq_sb[:, kd, 128:192],
                    rhs=hT[:, kd, csl],
                    start=(kd == 0), stop=(kd == KD - 1),
                    tile_position=(0, 0),
                )
                nc.tensor.matmul(
                    ps_d[64:128, :], lhsT=wk_sb[:, kd, 128:192],
                    rhs=hT[:, kd, csl],
                    start=(kd == 0), stop=(kd == KD - 1),
                    tile_position=(0, 64),
                )
            nc.vector.tensor_scalar(
                out=QT[0:64, 1, csl], in0=ps_c[0:64, :],
                scalar1=bq_sb[0:64, 1:2], scalar2=None, op0=ALU.add)
            nc.vector.tensor_scalar(
                out=KT[0:64, 1, csl], in0=ps_d[64:128, :],
                scalar1=bk_sb[0:64, 1:2], scalar2=None, op0=ALU.add)
            for gt in range(4 * c, 4 * c + 4):
                ps = pmm.tile([128, 512], F32, tag="mm", name="ps_v")
                for kd in range(KD):
                    nc.tensor.matmul(
                        ps[:, 0:DLOC],
                        lhsT=hT[:, kd, ds(128 * gt, 128)],
                        rhs=wv_sb[:, kd, :],
                        start=(kd == 0), stop=(kd == KD - 1),
                    )
                nc.vector.tensor_tensor(
                    out=Vx[:, gt, :, 0:64],
                    in0=ps[:, 0:DLOC].rearrange("p (h c) -> p h c", c=64),
                    in1=bv_rep[:, :].rearrange("p (h c) -> p h c", c=64),
                    op=ALU.add,
                )
            if c == 0:
                for ko in range(2):
                    nc.sync.dma_start(out=wo_sb[:, ko, :], in_=wo_s[ko])
            if c == 1:
                for kd in range(KD):
                    nc.sync.dma_start(out=w1_sb[:, kd, :], in_=w1_e[kd])
                for kh in range(KH):
                    nc.sync.dma_start(out=w2_sb[:, kh, :], in_=w2_e[kh])

            # ---- attention block c (q-range [512c, 512c+512)) ----
            OT = outw.tile([128, 2, 512], BF, tag="OT", name="OT")
            # rows 64-127 of the h2 lhsT tile multiply zero-padded Wo rows,
            # but must be finite
            nc.gpsimd.memset(OT[64:128, 1, :], 0.0)
            nkt = 4 * c + 4
            for i in range(HPC):
                o_ps = po.tile([65, 512], F32, tag="po", name="o_ps")
                for kt in range(nkt):
                    j = kt - 4 * c
                    qo = 128 * j if j > 0 else 0
                    w = 512 - qo
                    ps_s = psc.tile([128, 512], F32, tag="sc", name="ps_s")
                    nc.tensor.matmul(
                        ps_s[:, 0:w],
                        lhsT=head_qk(KT, i, 128 * kt, 128),
                        rhs=head_qk(QT, i, 512 * c + qo, w),
                        start=True, stop=True,
                    )
                    ex = exps.tile([128, 512], BF, tag="ex", name="ex")
                    nc.scalar.activation(
                        out=ex[:, 0:w], in_=ps_s[:, 0:w],
                        func=AF.Exp, scale=SCALE)
                    if j >= 0:
                        nc.vector.tensor_tensor(
                            out=ex[:, 0:128], in0=ex[:, 0:128], in1=cmask,
                            op=ALU.mult)
                    nc.tensor.matmul(
                        o_ps[:, qo:512],
                        lhsT=Vx[:, kt, i, :],
                        rhs=ex[:, 0:w],
                        start=(kt == 0), stop=(kt == nkt - 1),
                    )
                # divide by the ones-column denominator: reciprocal row ->
                # bf16 -> rank-1 PE matmul broadcast across 64 partitions;
                # head 1 lands partition-shifted into rows 64-127 of OT
                rc1 = stats.tile([1, 512], F32, tag="rc1", name="rc1", bufs=2)
                nc.vector.reciprocal(out=rc1, in_=o_ps[64:65, :])
                rc1b = stats.tile([1, 512], BF, tag="rc1b", name="rc1b",
                                  bufs=2)
                nc.vector.tensor_copy(out=rc1b, in_=rc1)
                rcb = pmm.tile([64, 512], F32, tag="mm", name="rcb")
                nc.tensor.matmul(rcb, lhsT=ones_r, rhs=rc1b,
                                 start=True, stop=True)
                rcb_sb = stats.tile([64, 512], BF, tag="rcb_sb",
                                    name="rcb_sb", bufs=2)
                nc.scalar.copy(out=rcb_sb, in_=rcb)
                dst = (OT[0:64, 0, :], OT[64:128, 0, :], OT[0:64, 1, :])[i]
                nc.vector.tensor_tensor(
                    out=dst, in0=o_ps[0:64, :], in1=rcb_sb, op=ALU.mult)

            # ---- out-projection + chunked ReduceScatter ----
            for tq in range(4):
                ybf = outw.tile([128, D], BF, tag="ybf", name="ybf")
                for nsl, nsz in ((0, 512), (512, 256)):
                    ps_y = pmm.tile([128, 512], F32, tag="mm", name="ps_y")
                    for ko in range(2):
                        nc.tensor.matmul(
                            ps_y[:, 0:nsz],
                            lhsT=OT[:, ko, ds(128 * tq, 128)],
                            rhs=wo_sb[:, ko, ds(nsl, nsz)],
                            start=(ko == 0), stop=(ko == 1),
                        )
                    nc.vector.tensor_copy(out=ybf[:, ds(nsl, nsz)],
                                          in_=ps_y[:, 0:nsz])
                nc.sync.dma_start(out=cc_in[4 * c + tq], in_=ybf)
            nc.gpsimd.collective_compute(
                "ReduceScatter",
                ALU.add,
                replica_groups=[[0, 1, 2, 3], [4, 5, 6, 7]],
                ins=[cc_in[ds(4 * c, 4)]],
                outs=[cc_rs[ds(c, 1)]],
            )

            # ---- MLP for the previous chunk's shard tile ----
            if c >= 1:
                mlp_tile(c - 1)
        mlp_tile(NB - 1)

    return nc


def make_in_maps(x, Wq, Wk, Wv, Wo, W1, W2, g1, b1, g2, b2):
    """Host-side sharding: per-core input dicts (one NEFF, per-core data)."""
    x = np.ascontiguousarray(np.asarray(x, np.float32))
    g1 = np.asarray(g1, np.float32)
    b1 = np.asarray(b1, np.float32)
    g2 = np.asarray(g2, np.float32)
    b2 = np.asarray(b2, np.float32)
    Wq, Wk, Wv, Wo = (np.asarray(w, np.float32) for w in (Wq, Wk, Wv, Wo))
    W1, W2 = np.asarray(W1, np.float32), np.asarray(W2, np.float32)

    # fold LN gains into the weights; LN biases become per-output biases
    wq_g = g1[:, None] * Wq
    wk_g = g1[:, None] * Wk
    wv_g = g1[:, None] * Wv
    w1_g = g2[:, None] * W1
    bias_q = b1 @ Wq
    bias_k = b1 @ Wk
    bias_v = b1 @ Wv
    bias_1 = b2 @ W1

    w2_bf = W2.astype(BF_NP).reshape(KH, 128, D)
    w1_bf = w1_g.astype(BF_NP).reshape(KD, 128, DH)

    def pad_to(a, n):
        out = np.zeros((n,) + a.shape[1:], a.dtype)
        out[: a.shape[0]] = a
        return out

    in_maps = []
    for c in range(NCORES):
        b, r = divmod(c, GRP)
        hsl = slice(DLOC * r, DLOC * (r + 1))
        in_maps.append({
            "x_full": x[b].reshape(NT, 128, D).astype(BF_NP),
            "x_own": np.stack([x[b, 512 * k + 128 * r: 512 * k + 128 * (r + 1)]
                               for k in range(NSH)]),
            "wq_s": np.ascontiguousarray(wq_g[:, hsl]).astype(BF_NP)
                      .reshape(KD, 128, DLOC),
            "wk_s": np.ascontiguousarray(wk_g[:, hsl]).astype(BF_NP)
                      .reshape(KD, 128, DLOC),
            "wv_s": np.ascontiguousarray(wv_g[:, hsl]).astype(BF_NP)
                      .reshape(KD, 128, DLOC),
            "wo_s": pad_to(np.ascontiguousarray(Wo[hsl]), DPAD)
                      .astype(BF_NP).reshape(2, 128, D),
            "w1_e": w1_bf,
            "w2_e": w2_bf,
            "bq_s": pad_to(np.ascontiguousarray(bias_q[hsl]), DPAD)
                      .reshape(2, 128),
            "bk_s": pad_to(np.ascontiguousarray(bias_k[hsl]), DPAD)
                      .reshape(2, 128),
            "bv_s": np.ascontiguousarray(bias_v[hsl]),
            "b1_s": bias_1.astype(BF_NP),
        })
    return in_maps


def assemble_output(results):
    out = np.empty((B, T, D), np.float32)
    for core in range(NCORES):
        b, r = divmod(core, GRP)
        for c in range(NSH):
            out[b, 512 * c + 128 * r: 512 * c + 128 * (r + 1)] = \
                results[core]["y_out"][c]
    return out


_NC_CACHE = {}


def get_nc():
    if "nc" not in _NC_CACHE:
        _NC_CACHE["nc"] = build_nc()
    return _NC_CACHE["nc"]


def run(in_maps, **kwargs):
    nc = get_nc()
    if not nc.is_finalized():
        nc.finalize()
    return run_bass_kernel_spmd(nc, in_maps, list(range(NCORES)), **kwargs)


def kernel(**inputs):
    in_maps = make_in_maps(**inputs)
    res = run(in_maps)
    return assemble_output(res.results)


if __name__ == "__main__":
    nc = build_nc()
    print("built OK")


# revision 22
# speedup vs baseline: 1.3806x; 1.3806x over previous
"""Trainium2 Bass kernel for one pre-LN transformer block (B=2, T=2048, D=768,
H=12 causal attention + 4x MLP), sharded over 8 NeuronCores.

Sharding (SPMD, one NEFF for all cores):
  * 2 batch groups x 4 cores.  Within a group, attention is tensor-parallel
    over heads (3 heads/core, full 2048-token causal attention), producing a
    partial out-projection y_c.  One chunked ReduceScatter(add) per 512-token
    q-block both sums the head contributions and token-shards the result.
  * The MLP sublayer is token-parallel and pipelined: each core runs LN2 +
    MLP on each 128-token shard tile as soon as its ReduceScatter chunk
    lands, overlapping the remaining attention blocks.
  * LayerNorm gains/biases are folded into the weights host-side.  rstd is
    computed with a table-free Newton iteration (reciprocal + polish), so the
    scalar engine needs exactly one activation-table set (exp/relu/copy).

Matmul structure:
  * LN1 output is transposed 128x128-wise on the PE; all 6 transposes of a
    tile share one PSUM bank (start=True only on the first) and leave in a
    single wide copy.
  * Attention per q-block b (512 queries), per head: for each k-tile one
    wide scores matmul (K stationary), one exp() ACT, and one V-stationary
    matmul accumulating O^T[65, 512] (extra ones-column gives the softmax
    denominator).  The diagonal k-tile is truncated to the causal width and
    its first q-tile masked on the vector engine.
  * O^T is divided by the denominator and written into a merged [128, 2, 512]
    lhsT tile (head 1 lands partition-shifted to rows 64-127), so the out
    projection is 2 matmuls per 512-col output slice.
  * MLP up-projection is computed token-tile-wise with the weight as the
    moving operand (rhs width 512 regardless of token batching), relu on
    gpsimd, transposed on the PE, down-projection with aT stationary.

All matmuls run in bf16 (weights pre-cast on host) with fp32 PSUM
accumulation; layernorm, softmax normalization and residuals are fp32.
"""

import math
from contextlib import ExitStack

import ml_dtypes
import numpy as np

import concourse.bass as bass
import concourse.bacc as bacc_mod
import concourse.mybir as mybir
import concourse.tile as tile
from concourse.bass import ds
from concourse.bass_utils import run_bass_kernel_spmd
from concourse.masks import make_identity

B, T, D, H, HD = 2, 2048, 768, 12, 64
DH = 4 * D                  # 3072 mlp hidden
EPS = 1e-5
NCORES = 8
GRP = 4                     # cores per batch group
HPC = H // GRP              # 3 heads per core
DLOC = HPC * HD             # 192 local head dims
DPAD = 256                  # local head dims padded to 2x128
TSH = T // GRP              # 512-token shard for the MLP phase
NT = T // 128               # 16 token tiles per batch
NB = 4                      # q-blocks (512 queries each)
NSH = TSH // 128            # 4 shard tiles
KD = D // 128               # 6
KH = DH // 128              # 24
SCALE = HD ** -0.5

BF = mybir.dt.bfloat16
F32 = mybir.dt.float32
AF = mybir.ActivationFunctionType
ALU = mybir.AluOpType
BF_NP = ml_dtypes.bfloat16


def _newton_rstd(nc, pool, var_ap, rstd_out, n):
    """rstd_out[128, n] = (var + EPS) ** -0.5, table-free.

    y0 = 1/(v+eps) from the DVE reciprocal, then 4 Newton steps
    y <- y * (1.5 - 0.5 * (v+eps) * y^2).  Converges to <1e-6 rel for
    v in [0.2, 3]; our rows have v ~= 1."""
    t = pool.tile([128, n], F32, tag="nw_t", name="nw_t", bufs=2)
    u = pool.tile([128, n], F32, tag="nw_u", name="nw_u", bufs=2)
    nc.vector.tensor_scalar(out=t, in0=var_ap, scalar1=EPS, scalar2=None,
                            op0=ALU.add)
    nc.vector.reciprocal(out=rstd_out, in_=t)
    for _ in range(4):
        nc.vector.tensor_tensor(out=u, in0=rstd_out, in1=rstd_out, op=ALU.mult)
        nc.vector.tensor_tensor(out=u, in0=u, in1=t, op=ALU.mult)
        nc.vector.tensor_scalar(out=u, in0=u, scalar1=-0.5, scalar2=1.5,
                                op0=ALU.mult, op1=ALU.add)
        nc.vector.tensor_tensor(out=rstd_out, in0=rstd_out, in1=u, op=ALU.mult)


def build_nc(mlp_bias=True):
    nc = bacc_mod.Bacc(None, num_devices=NCORES)

    # ---- per-core external I/O (host does the slicing / padding) ----
    x_full = nc.dram_tensor("x_full", [NT, 128, D], BF, kind="ExternalInput")
    x_own = nc.dram_tensor("x_own", [NSH, 128, D], F32, kind="ExternalInput")
    wq_s = nc.dram_tensor("wq_s", [KD, 128, DLOC], BF, kind="ExternalInput")
    wk_s = nc.dram_tensor("wk_s", [KD, 128, DLOC], BF, kind="ExternalInput")
    wv_s = nc.dram_tensor("wv_s", [KD, 128, DLOC], BF, kind="ExternalInput")
    wo_s = nc.dram_tensor("wo_s", [2, 128, D], BF, kind="ExternalInput")
    w1_e = nc.dram_tensor("w1_e", [KD, 128, DH], BF, kind="ExternalInput")
    w2_e = nc.dram_tensor("w2_e", [KH, 128, D], BF, kind="ExternalInput")
    bq_s = nc.dram_tensor("bq_s", [2, 128], F32, kind="ExternalInput")
    bk_s = nc.dram_tensor("bk_s", [2, 128], F32, kind="ExternalInput")
    bv_s = nc.dram_tensor("bv_s", [DLOC], F32, kind="ExternalInput")
    b1_s = nc.dram_tensor("b1_s", [DH], BF, kind="ExternalInput")
    y_out = nc.dram_tensor("y_out", [NSH, 128, D], F32, kind="ExternalOutput")

    with tile.TileContext(nc) as tc, ExitStack() as ctx:
        # ---------------- pools ----------------
        consts = ctx.enter_context(tc.tile_pool(name="consts", bufs=1))
        state = ctx.enter_context(tc.tile_pool(name="state", bufs=1))
        dram = ctx.enter_context(
            tc.tile_pool(name="dram", bufs=1, space="DRAM"))
        stats = ctx.enter_context(tc.tile_pool(name="stats", bufs=4))
        work = ctx.enter_context(tc.tile_pool(name="work", bufs=3))
        exps = ctx.enter_context(tc.tile_pool(name="exps", bufs=3))
        outw = ctx.enter_context(tc.tile_pool(name="outw", bufs=2))
        # psum: mm 3 banks + scores 2x1 + O 2x1 = 7 banks (1 spare)
        pmm = ctx.enter_context(tc.tile_pool(name="pmm", bufs=3, space="PSUM"))
        psc = ctx.enter_context(tc.tile_pool(name="psc", bufs=2, space="PSUM"))
        po = ctx.enter_context(tc.tile_pool(name="po", bufs=2, space="PSUM"))

        # ---------------- constants ----------------
        ident = consts.tile([128, 128], BF, tag="ident")
        make_identity(nc, ident)
        # causal mask for diagonal tiles: cmask[k, q] = 1.0 iff q >= k
        cmask = consts.tile([128, 128], BF, tag="cmask")
        nc.vector.memset(cmask, 1.0)
        nc.gpsimd.affine_select(
            out=cmask, in_=cmask, compare_op=ALU.is_ge, fill=0.0, base=0,
            pattern=[[1, 128]], channel_multiplier=-1)
        ones_r = consts.tile([1, 64], BF, tag="ones_r")
        nc.vector.memset(ones_r, 1.0)
        bq_sb = consts.tile([128, 2], F32, tag="bq")
        bk_sb = consts.tile([128, 2], F32, tag="bk")
        for ko in range(2):
            nc.sync.dma_start(out=bq_sb[:, ko:ko + 1], in_=bq_s[ko, :, None])
            nc.sync.dma_start(out=bk_sb[:, ko:ko + 1], in_=bk_s[ko, :, None])
        bv_rep = consts.tile([128, DLOC], F32, tag="bv")
        bv_ap = bv_s[:]
        nc.sync.dma_start(
            out=bv_rep,
            in_=bass.AP(tensor=bv_ap.tensor, offset=bv_ap.offset,
                        ap=[[0, 128]] + list(bv_ap.ap)))
        b1_rep = consts.tile([128, DH], BF, tag="b1r")
        b1_ap = b1_s[:]
        nc.sync.dma_start(
            out=b1_rep,
            in_=bass.AP(tensor=b1_ap.tensor, offset=b1_ap.offset,
                        ap=[[0, 128]] + list(b1_ap.ap)))

        # ---------------- persistent state ----------------
        # Q/K with head i at (tile 64i//128, base 64i%128); tile1 rows 64-127
        # unused.
        QT = state.tile([128, 2, T], BF, tag="QT")
        KT = state.tile([128, 2, T], BF, tag="KT")
        Vx = state.tile([128, NT, HPC, 65], BF, tag="Vx")  # V + ones col
        hT = state.tile([128, KD, T], BF, tag="hT")
        xr = state.tile([128, NSH, D], BF, tag="xr")  # x + attn residual
        # weights (bf16, host-cast)
        wq_sb = state.tile([128, KD, DLOC], BF, tag="wq")
        wk_sb = state.tile([128, KD, DLOC], BF, tag="wk")
        wv_sb = state.tile([128, KD, DLOC], BF, tag="wv")
        wo_sb = state.tile([128, 2, D], BF, tag="wo")
        w1_sb = state.tile([128, KD, DH], BF, tag="w1")
        w2_sb = state.tile([128, KH, D], BF, tag="w2")
        mv1 = consts.tile([128, NT, 2], F32, tag="mv1")
        rstd1 = consts.tile([128, NT], F32, tag="rstd1")
        mv2 = consts.tile([128, NSH, 2], F32, tag="mv2")
        rstd2 = consts.tile([128, NSH], F32, tag="rstd2")
        # ones columns of extended-V (softmax denominators for free)
        nc.vector.memset(Vx[:, :, :, 64:65], 1.0)

        # collective buffers
        cc_in = dram.tile([NT, 128, D], BF, tag="cc_in")
        cc_rs = dram.tile([NSH, 128, D], BF, tag="cc_rs")

        def head_qk(t_sb, i, col, n):
            kd_i, base = (64 * i) // 128, (64 * i) % 128
            return t_sb[base:base + 64, kd_i, ds(col, n)]

        def ln_tile(x_t, mv_out):
            st = stats.tile([128, 3, 6], F32, tag="st", name="ln_st")
            for s in range(3):
                nc.vector.bn_stats(out=st[:, s, :], in_=x_t[:, ds(256 * s, 256)])
            nc.vector.bn_aggr(out=mv_out, in_=st)

        def transpose6(src_bf, dst_ap, on_scalar):
            """6 PE transposes of a [128, 768] bf16 tile through one PSUM
            bank, then one wide copy to dst_ap ([128, 6, 128] view)."""
            ps_tr = pmm.tile([128, KD, 128], BF, tag="mm", name="ps_tr")
            for kd in range(KD):
                nc.tensor.matmul(
                    ps_tr[:, kd, :], lhsT=src_bf[:, ds(128 * kd, 128)],
                    rhs=ident, is_transpose=True,
                    start=(kd == 0), stop=True,
                    skip_group_check=(kd != 0))
            if on_scalar:
                nc.scalar.copy(out=dst_ap, in_=ps_tr)
            else:
                nc.vector.tensor_copy(out=dst_ap, in_=ps_tr)

        def mlp_tile(tt):
            """Residual + LN2 + MLP for shard tile tt (after RS chunk tt)."""
            y_sb = work.tile([128, D], BF, tag="y_sb", name="y_sb", bufs=2)
            nc.sync.dma_start(out=y_sb, in_=cc_rs[tt])
            xo = work.tile([128, D], F32, tag="xo", name="xo", bufs=2)
            nc.sync.dma_start(out=xo, in_=x_own[tt])
            nc.vector.tensor_tensor(
                out=xr[:, tt, :], in0=xo, in1=y_sb, op=ALU.add)
            ln_tile(xr[:, tt, :], mv2[:, tt, :])
            _newton_rstd(nc, stats, mv2[:, tt, 1:2], rstd2[:, tt:tt + 1], 1)
            h2 = work.tile([128, D], BF, tag="h_bf", name="h2", bufs=2)
            nc.vector.tensor_scalar(
                out=h2, in0=xr[:, tt, :], scalar1=mv2[:, tt, 0:1],
                scalar2=rstd2[:, tt:tt + 1], op0=ALU.subtract, op1=ALU.mult)
            h2t = work.tile([128, KD, 128], BF, tag="h2t", name="h2t", bufs=2)
            transpose6(h2, h2t, True)
            # up-projection: weight is the moving operand (512-wide rhs)
            aT_t = work.tile([128, KH, 128], BF, tag="aT_t", name="aT_t",
                             bufs=2)
            for sl in range(KH // 4):
                ps_up = pmm.tile([128, 512], F32, tag="mm", name="ps_up")
                for kd in range(KD):
                    nc.tensor.matmul(
                        ps_up,
                        lhsT=h2t[:, kd, :],
                        rhs=w1_sb[:, kd, ds(512 * sl, 512)],
                        start=(kd == 0), stop=(kd == KD - 1),
                    )
                a_bf = work.tile([128, 512], BF, tag="a_bf", name="a_bf",
                                 bufs=2)
                if mlp_bias:
                    nc.vector.tensor_tensor(
                        out=a_bf, in0=ps_up, in1=b1_rep[:, ds(512 * sl, 512)],
                        op=ALU.add)
                    nc.scalar.activation(out=a_bf, in_=a_bf, func=AF.Relu)
                else:
                    # bias is exactly zero: relu straight off PSUM
                    nc.scalar.activation(out=a_bf, in_=ps_up, func=AF.Relu)
                # transpose the 4 kh-tiles of this slice through one bank
                ps_at = pmm.tile([128, 4, 128], BF, tag="mm", name="ps_at")
                for k4 in range(4):
                    nc.tensor.matmul(
                        ps_at[:, k4, :], lhsT=a_bf[:, ds(128 * k4, 128)],
                        rhs=ident, is_transpose=True,
                        start=(k4 == 0), stop=True,
                        skip_group_check=(k4 != 0))
                nc.vector.tensor_copy(out=aT_t[:, ds(4 * sl, 4), :], in_=ps_at)
            # down-projection + final residual
            out_t = work.tile([128, D], F32, tag="out_t", name="out_t",
                              bufs=2)
            for nsl, nsz in ((0, 512), (512, 256)):
                ps_dn = pmm.tile([128, 512], F32, tag="mm", name="ps_dn")
                for dh in range(KH):
                    nc.tensor.matmul(
                        ps_dn[:, 0:nsz],
                        lhsT=aT_t[:, dh, :],
                        rhs=w2_sb[:, dh, ds(nsl, nsz)],
                        start=(dh == 0), stop=(dh == KH - 1),
                    )
                nc.vector.tensor_tensor(
                    out=out_t[:, ds(nsl, nsz)], in0=ps_dn[:, 0:nsz],
                    in1=xr[:, tt, ds(nsl, nsz)], op=ALU.add)
            nc.sync.dma_start(out=y_out[tt], in_=out_t)

        # ================= main pipeline =================
        for c in range(NB):
            # ---- LN1 chunk c: dma -> stats -> rstd -> normalize+transpose
            x_ts = []
            for j, gt in enumerate(range(4 * c, 4 * c + 4)):
                x_t = work.tile([128, D], BF, tag="x_t", name="x_t", bufs=4)
                nc.sync.dma_start(out=x_t, in_=x_full[gt])
                x_ts.append(x_t)
            if c == 0:
                for kd in range(KD):
                    nc.sync.dma_start(out=wk_sb[:, kd, :], in_=wk_s[kd])
                    nc.sync.dma_start(out=wq_sb[:, kd, :], in_=wq_s[kd])
                    nc.sync.dma_start(out=wv_sb[:, kd, :], in_=wv_s[kd])
            for j, gt in enumerate(range(4 * c, 4 * c + 4)):
                ln_tile(x_ts[j], mv1[:, gt, :])
            _newton_rstd(nc, stats, mv1[:, ds(4 * c, 4), 1],
                         rstd1[:, ds(4 * c, 4)], 4)
            for j, gt in enumerate(range(4 * c, 4 * c + 4)):
                h_bf = work.tile([128, D], BF, tag="h_bf", name="h_bf",
                                 bufs=2)
                nc.vector.tensor_scalar(
                    out=h_bf, in0=x_ts[j], scalar1=mv1[:, gt, 0:1],
                    scalar2=rstd1[:, gt:gt + 1], op0=ALU.subtract, op1=ALU.mult)
                transpose6(h_bf, hT[:, :, ds(128 * gt, 128)], gt % 2 == 0)

            # ---- QKV chunk c ----
            csl = ds(512 * c, 512)
            for w_sb, t_sb, b_sb in ((wk_sb, KT, bk_sb), (wq_sb, QT, bq_sb)):
                ps = pmm.tile([128, 512], F32, tag="mm", name="ps_qk")
                for kd in range(KD):
                    nc.tensor.matmul(
                        ps, lhsT=w_sb[:, kd, 0:128], rhs=hT[:, kd, csl],
                        start=(kd == 0), stop=(kd == KD - 1),
                    )
                nc.vector.tensor_scalar(
                    out=t_sb[:, 0, csl], in0=ps, scalar1=b_sb[:, 0:1],
                    scalar2=None, op0=ALU.add)
            # head-2 Q and K col-tiled into concurrent halves of the PE
            ps_c = pmm.tile([128, 512], F32, tag="mm", name="ps_c")
            ps_d = pmm.tile([128, 512], F32, tag="mm", name="ps_d")
            for kd in range(KD):
                nc.tensor.matmul(
                    ps_c[0:64, :], lhsT=wq_sb[:, kd, 128:192],
                    rhs=hT[:, kd, csl],
                    start=(kd == 0), stop=(kd == KD - 1),
                    tile_position=(0, 0),
                )
                nc.tensor.matmul(
                    ps_d[64:128, :], lhsT=wk_sb[:, kd, 128:192],
                    rhs=hT[:, kd, csl],
                    start=(kd == 0), stop=(kd == KD - 1),
                    tile_position=(0, 64),
                )
            nc.vector.tensor_scalar(
                out=QT[0:64, 1, csl], in0=ps_c[0:64, :],
                scalar1=bq_sb[0:64, 1:2], scalar2=None, op0=ALU.add)
            nc.vector.tensor_scalar(
                out=KT[0:64, 1, csl], in0=ps_d[64:128, :],
                scalar1=bk_sb[0:64, 1:2], scalar2=None, op0=ALU.add)
            for gt in range(4 * c, 4 * c + 4):
                ps = pmm.tile([128, 512], F32, tag="mm", name="ps_v")
                for kd in range(KD):
                    nc.tensor.matmul(
                        ps[:, 0:DLOC],
                        lhsT=hT[:, kd, ds(128 * gt, 128)],
                        rhs=wv_sb[:, kd, :],
                        start=(kd == 0), stop=(kd == KD - 1),
                    )
                nc.vector.tensor_tensor(
                    out=Vx[:, gt, :, 0:64],
                    in0=ps[:, 0:DLOC].rearrange("p (h c) -> p h c", c=64),
                    in1=bv_rep[:, :].rearrange("p (h c) -> p h c", c=64),
                    op=ALU.add,
                )
            if c == 0:
                for ko in range(2):
                    nc.sync.dma_start(out=wo_sb[:, ko, :], in_=wo_s[ko])
            if c == 1:
                for kd in range(KD):
                    nc.sync.dma_start(out=w1_sb[:, kd, :], in_=w1_e[kd])
                for kh in range(KH):
                    nc.sync.dma_start(out=w2_sb[:, kh, :], in_=w2_e[kh])

            # ---- attention block c (q-range [512c, 512c+512)) ----
            OT = outw.tile([128, 2, 512], BF, tag="OT", name="OT")
            # rows 64-127 of the h2 lhsT tile multiply zero-padded Wo rows,
            # but must be finite
            nc.gpsimd.memset(OT[64:128, 1, :], 0.0)
            nkt = 4 * c + 4
            for i in range(HPC):
                o_ps = po.tile([65, 512], F32, tag="po", name="o_ps")
                for kt in range(nkt):
                    j = kt - 4 * c
                    qo = 128 * j if j > 0 else 0
                    w = 512 - qo
                    ps_s = psc.tile([128, 512], F32, tag="sc", name="ps_s")
                    nc.tensor.matmul(
                        ps_s[:, 0:w],
                        lhsT=head_qk(KT, i, 128 * kt, 128),
                        rhs=head_qk(QT, i, 512 * c + qo, w),
                        start=True, stop=True,
                    )
                    ex = exps.tile([128, 512], BF, tag="ex", name="ex")
                    nc.scalar.activation(
                        out=ex[:, 0:w], in_=ps_s[:, 0:w],
                        func=AF.Exp, scale=SCALE)
                    if j >= 0:
                        nc.vector.tensor_tensor(
                            out=ex[:, 0:128], in0=ex[:, 0:128], in1=cmask,
                            op=ALU.mult)
                    nc.tensor.matmul(
                        o_ps[:, qo:512],
                        lhsT=Vx[:, kt, i, :],
                        rhs=ex[:, 0:w],
                        start=(kt == 0), stop=(kt == nkt - 1),
                    )
                # divide by the ones-column denominator: bf16-cast the denom
                # row on the scalar engine, rank-1 PE matmul broadcasts it to
                # 64 partitions, wide 64-lane reciprocal, multiply.  Head 1
                # lands partition-shifted into rows 64-127 of the merged OT.
                dn1 = stats.tile([1, 512], BF, tag="dn1", name="dn1", bufs=2)
                nc.scalar.copy(out=dn1, in_=o_ps[64:65, :])
                dnb = pmm.tile([64, 512], F32, tag="mm", name="dnb")
                nc.tensor.matmul(dnb, lhsT=ones_r, rhs=dn1,
                                 start=True, stop=True)
                rcb_sb = stats.tile([64, 512], F32, tag="rcb_sb",
                                    name="rcb_sb", bufs=2)
                nc.vector.reciprocal(out=rcb_sb, in_=dnb)
                dst = (OT[0:64, 0, :], OT[64:128, 0, :], OT[0:64, 1, :])[i]
                nc.vector.tensor_tensor(
                    out=dst, in0=o_ps[0:64, :], in1=rcb_sb, op=ALU.mult)

            # ---- out-projection + chunked ReduceScatter ----
            for tq in range(4):
                ybf = outw.tile([128, D], BF, tag="ybf", name="ybf")
                for nsl, nsz in ((0, 512), (512, 256)):
                    ps_y = pmm.tile([128, 512], F32, tag="mm", name="ps_y")
                    for ko in range(2):
                        nc.tensor.matmul(
                            ps_y[:, 0:nsz],
                            lhsT=OT[:, ko, ds(128 * tq, 128)],
                            rhs=wo_sb[:, ko, ds(nsl, nsz)],
                            start=(ko == 0), stop=(ko == 1),
                        )
                    nc.vector.tensor_copy(out=ybf[:, ds(nsl, nsz)],
                                          in_=ps_y[:, 0:nsz])
                nc.sync.dma_start(out=cc_in[4 * c + tq], in_=ybf)
            nc.gpsimd.collective_compute(
                "ReduceScatter",
                ALU.add,
                replica_groups=[[0, 1, 2, 3], [4, 5, 6, 7]],
                ins=[cc_in[ds(4 * c, 4)]],
                outs=[cc_rs[ds(c, 1)]],
            )

            # ---- MLP for the previous chunk's shard tile ----
            if c >= 1:
                mlp_tile(c - 1)
        mlp_tile(NB - 1)

    return nc


def make_in_maps(x, Wq, Wk, Wv, Wo, W1, W2, g1, b1, g2, b2):
    """Host-side sharding: per-core input dicts (one NEFF, per-core data)."""
    x = np.ascontiguousarray(np.asarray(x, np.float32))
    g1 = np.asarray(g1, np.float32)
    b1 = np.asarray(b1, np.float32)
    g2 = np.asarray(g2, np.float32)
    b2 = np.asarray(b2, np.float32)
    Wq, Wk, Wv, Wo = (np.asarray(w, np.float32) for w in (Wq, Wk, Wv, Wo))
    W1, W2 = np.asarray(W1, np.float32), np.asarray(W2, np.float32)

    # fold LN gains into the weights; LN biases become per-output biases
    wq_g = g1[:, None] * Wq
    wk_g = g1[:, None] * Wk
    wv_g = g1[:, None] * Wv
    w1_g = g2[:, None] * W1
    bias_q = b1 @ Wq
    bias_k = b1 @ Wk
    bias_v = b1 @ Wv
    bias_1 = b2 @ W1

    w2_bf = W2.astype(BF_NP).reshape(KH, 128, D)
    w1_bf = w1_g.astype(BF_NP).reshape(KD, 128, DH)

    def pad_to(a, n):
        out = np.zeros((n,) + a.shape[1:], a.dtype)
        out[: a.shape[0]] = a
        return out

    in_maps = []
    for c in range(NCORES):
        b, r = divmod(c, GRP)
        hsl = slice(DLOC * r, DLOC * (r + 1))
        in_maps.append({
            "x_full": x[b].reshape(NT, 128, D).astype(BF_NP),
            "x_own": np.stack([x[b, 512 * k + 128 * r: 512 * k + 128 * (r + 1)]
                               for k in range(NSH)]),
            "wq_s": np.ascontiguousarray(wq_g[:, hsl]).astype(BF_NP)
                      .reshape(KD, 128, DLOC),
            "wk_s": np.ascontiguousarray(wk_g[:, hsl]).astype(BF_NP)
                      .reshape(KD, 128, DLOC),
            "wv_s": np.ascontiguousarray(wv_g[:, hsl]).astype(BF_NP)
                      .reshape(KD, 128, DLOC),
            "wo_s": pad_to(np.ascontiguousarray(Wo[hsl]), DPAD)
                      .astype(BF_NP).reshape(2, 128, D),
            "w1_e": w1_bf,
            "w2_e": w2_bf,
            "bq_s": pad_to(np.ascontiguousarray(bias_q[hsl]), DPAD)
                      .reshape(2, 128),
            "bk_s": pad_to(np.ascontiguousarray(bias_k[hsl]), DPAD)
                      .reshape(2, 128),
            "bv_s": np.ascontiguousarray(bias_v[hsl]),
            "b1_s": bias_1.astype(BF_NP),
        })
    return in_maps


def assemble_output(results):
    out = np.empty((B, T, D), np.float32)
    for core in range(NCORES):
        b, r = divmod(core, GRP)
        for c in range(NSH):
            out[b, 512 * c + 128 * r: 512 * c + 128 * (r + 1)] = \
                results[core]["y_out"][c]
    return out


_NC_CACHE = {}


def get_nc(mlp_bias=True):
    if mlp_bias not in _NC_CACHE:
        _NC_CACHE[mlp_bias] = build_nc(mlp_bias)
    return _NC_CACHE[mlp_bias]


def run(in_maps, **kwargs):
    mlp_bias = bool(np.any(np.asarray(in_maps[0]["b1_s"], np.float32)))
    nc = get_nc(mlp_bias)
    if not nc.is_finalized():
        nc.finalize()
    return run_bass_kernel_spmd(nc, in_maps, list(range(NCORES)), **kwargs)


def kernel(**inputs):
    in_maps = make_in_maps(**inputs)
    res = run(in_maps)
    return assemble_output(res.results)


if __name__ == "__main__":
    nc = build_nc()
    print("built OK")


# revision 28
# speedup vs baseline: 1.5912x; 1.1526x over previous
"""Trainium2 Bass kernel for one pre-LN transformer block (B=2, T=2048, D=768,
H=12 causal attention + 4x MLP), sharded over 8 NeuronCores.

Sharding (SPMD, one NEFF for all cores):
  * 2 batch groups x 4 cores.  Within a group, attention is tensor-parallel
    over heads (3 heads/core, full 2048-token causal attention), producing a
    partial out-projection y_c.  One chunked ReduceScatter(add) per 512-token
    q-block both sums the head contributions and token-shards the result.
  * The MLP sublayer is token-parallel and pipelined: each core runs LN2 +
    MLP on each 128-token shard tile as soon as its ReduceScatter chunk
    lands, overlapping the remaining attention blocks.
  * LayerNorm gains/biases are folded into the weights host-side.  rstd is
    computed with a table-free Newton iteration (reciprocal + polish), so the
    scalar engine needs exactly one activation-table set (exp/relu/copy).

Matmul structure:
  * LN1 output is transposed 128x128-wise on the PE; all 6 transposes of a
    tile share one PSUM bank (start=True only on the first) and leave in a
    single wide copy.
  * Attention per q-block b (512 queries), per head: for each k-tile one
    wide scores matmul (K stationary), one exp() ACT, and one V-stationary
    matmul accumulating O^T[65, 512] (extra ones-column gives the softmax
    denominator).  The diagonal k-tile is truncated to the causal width and
    its first q-tile masked on the vector engine.
  * O^T is divided by the denominator and written into a merged [128, 2, 512]
    lhsT tile (head 1 lands partition-shifted to rows 64-127), so the out
    projection is 2 matmuls per 512-col output slice.
  * MLP up-projection is computed token-tile-wise with the weight as the
    moving operand (rhs width 512 regardless of token batching), relu on
    gpsimd, transposed on the PE, down-projection with aT stationary.

All matmuls run in bf16 (weights pre-cast on host) with fp32 PSUM
accumulation; layernorm, softmax normalization and residuals are fp32.
"""

import math
from contextlib import ExitStack

import ml_dtypes
import numpy as np

import concourse.bass as bass
import concourse.bacc as bacc_mod
import concourse.mybir as mybir
import concourse.tile as tile
from concourse.bass import ds
from concourse.bass_utils import run_bass_kernel_spmd
from concourse.masks import make_identity

B, T, D, H, HD = 2, 2048, 768, 12, 64
DH = 4 * D                  # 3072 mlp hidden
EPS = 1e-5
NCORES = 8
GRP = 4                     # cores per batch group
HPC = H // GRP              # 3 heads per core
DLOC = HPC * HD             # 192 local head dims
DPAD = 256                  # local head dims padded to 2x128
TSH = T // GRP              # 512-token shard for the MLP phase
NT = T // 128               # 16 token tiles per batch
NB = 4                      # q-blocks (512 queries each)
NSH = TSH // 128            # 4 shard tiles
KD = D // 128               # 6
KH = DH // 128              # 24
SCALE = HD ** -0.5

BF = mybir.dt.bfloat16
F32 = mybir.dt.float32
AF = mybir.ActivationFunctionType
ALU = mybir.AluOpType
BF_NP = ml_dtypes.bfloat16


def _newton_rstd(nc, pool, var_ap, rstd_out, n):
    """rstd_out[128, n] = (var + EPS) ** -0.5, table-free.

    y0 = 1/(v+eps) from the DVE reciprocal, then 4 Newton steps
    y <- y * (1.5 - 0.5 * (v+eps) * y^2).  Converges to <1e-6 rel for
    v in [0.2, 3]; our rows have v ~= 1."""
    t = pool.tile([128, n], F32, tag="nw_t", name="nw_t", bufs=2)
    u = pool.tile([128, n], F32, tag="nw_u", name="nw_u", bufs=2)
    nc.vector.tensor_scalar(out=t, in0=var_ap, scalar1=EPS, scalar2=None,
                            op0=ALU.add)
    nc.vector.reciprocal(out=rstd_out, in_=t)
    for _ in range(4):
        nc.vector.tensor_tensor(out=u, in0=rstd_out, in1=rstd_out, op=ALU.mult)
        nc.vector.tensor_tensor(out=u, in0=u, in1=t, op=ALU.mult)
        nc.vector.tensor_scalar(out=u, in0=u, scalar1=-0.5, scalar2=1.5,
                                op0=ALU.mult, op1=ALU.add)
        nc.vector.tensor_tensor(out=rstd_out, in0=rstd_out, in1=u, op=ALU.mult)


def build_nc(mlp_bias=True):
    nc = bacc_mod.Bacc(None, num_devices=NCORES)

    # ---- per-core external I/O (host does the slicing / padding) ----
    x_full = nc.dram_tensor("x_full", [NT, 128, D], BF, kind="ExternalInput")
    x_own = nc.dram_tensor("x_own", [NSH, 128, D], F32, kind="ExternalInput")
    wq_s = nc.dram_tensor("wq_s", [KD, 128, DLOC], BF, kind="ExternalInput")
    wk_s = nc.dram_tensor("wk_s", [KD, 128, DLOC], BF, kind="ExternalInput")
    wv_s = nc.dram_tensor("wv_s", [KD, 128, DLOC], BF, kind="ExternalInput")
    wo_s = nc.dram_tensor("wo_s", [2, 128, D], BF, kind="ExternalInput")
    w1_e = nc.dram_tensor("w1_e", [KD, 128, DH], BF, kind="ExternalInput")
    w2_e = nc.dram_tensor("w2_e", [KH, 128, D], BF, kind="ExternalInput")
    bq_s = nc.dram_tensor("bq_s", [2, 128], F32, kind="ExternalInput")
    bk_s = nc.dram_tensor("bk_s", [2, 128], F32, kind="ExternalInput")
    bv_s = nc.dram_tensor("bv_s", [DLOC], F32, kind="ExternalInput")
    b1_s = nc.dram_tensor("b1_s", [DH], BF, kind="ExternalInput")
    y_out = nc.dram_tensor("y_out", [NSH, 128, D], F32, kind="ExternalOutput")

    with tile.TileContext(nc) as tc, ExitStack() as ctx:
        # ---------------- pools ----------------
        consts = ctx.enter_context(tc.tile_pool(name="consts", bufs=1))
        state = ctx.enter_context(tc.tile_pool(name="state", bufs=1))
        dram = ctx.enter_context(
            tc.tile_pool(name="dram", bufs=1, space="DRAM"))
        stats = ctx.enter_context(tc.tile_pool(name="stats", bufs=4))
        work = ctx.enter_context(tc.tile_pool(name="work", bufs=3))
        exps = ctx.enter_context(tc.tile_pool(name="exps", bufs=3))
        outw = ctx.enter_context(tc.tile_pool(name="outw", bufs=2))
        # psum: mm 2 banks + scores 2x2 (head-paired) + O 2x1 = 8 banks
        pmm = ctx.enter_context(tc.tile_pool(name="pmm", bufs=2, space="PSUM"))
        psc = ctx.enter_context(tc.tile_pool(name="psc", bufs=2, space="PSUM"))
        po = ctx.enter_context(tc.tile_pool(name="po", bufs=2, space="PSUM"))

        # ---------------- constants ----------------
        ident = consts.tile([128, 128], BF, tag="ident")
        make_identity(nc, ident)
        # causal mask for diagonal tiles: cmask[k, q] = 1.0 iff q >= k
        cmask = consts.tile([128, 128], BF, tag="cmask")
        nc.vector.memset(cmask, 1.0)
        nc.gpsimd.affine_select(
            out=cmask, in_=cmask, compare_op=ALU.is_ge, fill=0.0, base=0,
            pattern=[[1, 128]], channel_multiplier=-1)
        ones_r = consts.tile([1, 64], BF, tag="ones_r")
        nc.vector.memset(ones_r, 1.0)
        bq_sb = consts.tile([128, 2], F32, tag="bq")
        bk_sb = consts.tile([128, 2], F32, tag="bk")
        for ko in range(2):
            nc.sync.dma_start(out=bq_sb[:, ko:ko + 1], in_=bq_s[ko, :, None])
            nc.sync.dma_start(out=bk_sb[:, ko:ko + 1], in_=bk_s[ko, :, None])
        bv_rep = consts.tile([128, DLOC], F32, tag="bv")
        bv_ap = bv_s[:]
        nc.sync.dma_start(
            out=bv_rep,
            in_=bass.AP(tensor=bv_ap.tensor, offset=bv_ap.offset,
                        ap=[[0, 128]] + list(bv_ap.ap)))
        if mlp_bias:
            b1_rep = consts.tile([128, DH], BF, tag="b1r")
            b1_ap = b1_s[:]
            nc.sync.dma_start(
                out=b1_rep,
                in_=bass.AP(tensor=b1_ap.tensor, offset=b1_ap.offset,
                            ap=[[0, 128]] + list(b1_ap.ap)))

        # ---------------- persistent state ----------------
        # Q/K with head i at (tile 64i//128, base 64i%128); tile1 rows 64-127
        # unused.
        QT = state.tile([128, 2, T], BF, tag="QT")
        KT = state.tile([128, 2, T], BF, tag="KT")
        Vx = state.tile([128, NT, HPC, 65], BF, tag="Vx")  # V + ones col
        hT = state.tile([128, KD, T], BF, tag="hT")
        xr = state.tile([128, NSH, D], BF, tag="xr")  # x + attn residual
        # weights (bf16, host-cast)
        wq_sb = state.tile([128, KD, DLOC], BF, tag="wq")
        wk_sb = state.tile([128, KD, DLOC], BF, tag="wk")
        wv_sb = state.tile([128, KD, DLOC], BF, tag="wv")
        wo_sb = state.tile([128, 2, D], BF, tag="wo")
        w1_sb = state.tile([128, KD, DH], BF, tag="w1")
        w2_sb = state.tile([128, KH, D], BF, tag="w2")
        mv1 = consts.tile([128, NT, 2], F32, tag="mv1")
        rstd1 = consts.tile([128, NT], F32, tag="rstd1")
        mv2 = consts.tile([128, NSH, 2], F32, tag="mv2")
        rstd2 = consts.tile([128, NSH], F32, tag="rstd2")
        # ones columns of extended-V (softmax denominators for free)
        nc.vector.memset(Vx[:, :, :, 64:65], 1.0)

        # collective buffers
        cc_in = dram.tile([NT, 128, D], BF, tag="cc_in")
        cc_rs = dram.tile([NSH, 128, D], BF, tag="cc_rs")

        def head_qk(t_sb, i, col, n):
            kd_i, base = (64 * i) // 128, (64 * i) % 128
            return t_sb[base:base + 64, kd_i, ds(col, n)]

        def ln_tile(x_t, mv_out):
            st = stats.tile([128, 3, 6], F32, tag="st", name="ln_st")
            for s in range(3):
                nc.vector.bn_stats(out=st[:, s, :], in_=x_t[:, ds(256 * s, 256)])
            nc.vector.bn_aggr(out=mv_out, in_=st)

        def transpose6(src_bf, dst_ap, on_scalar):
            """6 PE transposes of a [128, 768] bf16 tile through one PSUM
            bank, then one wide copy to dst_ap ([128, 6, 128] view)."""
            ps_tr = pmm.tile([128, KD, 128], BF, tag="mm", name="ps_tr")
            for kd in range(KD):
                nc.tensor.matmul(
                    ps_tr[:, kd, :], lhsT=src_bf[:, ds(128 * kd, 128)],
                    rhs=ident, is_transpose=True,
                    start=(kd == 0), stop=True,
                    skip_group_check=(kd != 0))
            if on_scalar:
                nc.scalar.copy(out=dst_ap, in_=ps_tr)
            else:
                nc.vector.tensor_copy(out=dst_ap, in_=ps_tr)

        def mlp_tile(tt):
            """Residual + LN2 + MLP for shard tile tt (after RS chunk tt)."""
            y_sb = work.tile([128, D], BF, tag="y_sb", name="y_sb", bufs=2)
            nc.sync.dma_start(out=y_sb, in_=cc_rs[tt])
            xo = work.tile([128, D], F32, tag="xo", name="xo", bufs=2)
            nc.sync.dma_start(out=xo, in_=x_own[tt])
            nc.vector.tensor_tensor(
                out=xr[:, tt, :], in0=xo, in1=y_sb, op=ALU.add)
            ln_tile(xr[:, tt, :], mv2[:, tt, :])
            _newton_rstd(nc, stats, mv2[:, tt, 1:2], rstd2[:, tt:tt + 1], 1)
            h2 = work.tile([128, D], BF, tag="h_bf", name="h2", bufs=2)
            nc.vector.tensor_scalar(
                out=h2, in0=xr[:, tt, :], scalar1=mv2[:, tt, 0:1],
                scalar2=rstd2[:, tt:tt + 1], op0=ALU.subtract, op1=ALU.mult)
            h2t = work.tile([128, KD, 128], BF, tag="h2t", name="h2t", bufs=2)
            transpose6(h2, h2t, True)
            # up-projection: weight is the moving operand (512-wide rhs)
            aT_t = work.tile([128, KH, 128], BF, tag="aT_t", name="aT_t",
                             bufs=2)
            for sl in range(KH // 4):
                ps_up = pmm.tile([128, 512], F32, tag="mm", name="ps_up")
                for kd in range(KD):
                    nc.tensor.matmul(
                        ps_up,
                        lhsT=h2t[:, kd, :],
                        rhs=w1_sb[:, kd, ds(512 * sl, 512)],
                        start=(kd == 0), stop=(kd == KD - 1),
                    )
                a_bf = work.tile([128, 512], BF, tag="a_bf", name="a_bf",
                                 bufs=2)
                if mlp_bias:
                    nc.vector.tensor_tensor(
                        out=a_bf, in0=ps_up, in1=b1_rep[:, ds(512 * sl, 512)],
                        op=ALU.add)
                    nc.scalar.activation(out=a_bf, in_=a_bf, func=AF.Relu)
                else:
                    # bias is exactly zero: relu straight off PSUM
                    nc.scalar.activation(out=a_bf, in_=ps_up, func=AF.Relu)
                # transpose the 4 kh-tiles of this slice through one bank
                ps_at = pmm.tile([128, 4, 128], BF, tag="mm", name="ps_at")
                for k4 in range(4):
                    nc.tensor.matmul(
                        ps_at[:, k4, :], lhsT=a_bf[:, ds(128 * k4, 128)],
                        rhs=ident, is_transpose=True,
                        start=(k4 == 0), stop=True,
                        skip_group_check=(k4 != 0))
                nc.vector.tensor_copy(out=aT_t[:, ds(4 * sl, 4), :], in_=ps_at)
            # down-projection + final residual
            out_t = work.tile([128, D], F32, tag="out_t", name="out_t",
                              bufs=2)
            for nsl, nsz in ((0, 512), (512, 256)):
                ps_dn = pmm.tile([128, 512], F32, tag="mm", name="ps_dn")
                for dh in range(KH):
                    nc.tensor.matmul(
                        ps_dn[:, 0:nsz],
                        lhsT=aT_t[:, dh, :],
                        rhs=w2_sb[:, dh, ds(nsl, nsz)],
                        start=(dh == 0), stop=(dh == KH - 1),
                    )
                nc.vector.tensor_tensor(
                    out=out_t[:, ds(nsl, nsz)], in0=ps_dn[:, 0:nsz],
                    in1=xr[:, tt, ds(nsl, nsz)], op=ALU.add)
            nc.sync.dma_start(out=y_out[tt], in_=out_t)

        # ================= main pipeline =================
        for c in range(NB):
            # ---- LN1 chunk c: dma -> stats -> rstd -> normalize+transpose
            x_ts = []
            for j, gt in enumerate(range(4 * c, 4 * c + 4)):
                x_t = work.tile([128, D], BF, tag="x_t", name="x_t", bufs=4)
                nc.sync.dma_start(out=x_t, in_=x_full[gt])
                x_ts.append(x_t)
            if c == 0:
                for kd in range(KD):
                    nc.sync.dma_start(out=wk_sb[:, kd, :], in_=wk_s[kd])
                    nc.sync.dma_start(out=wq_sb[:, kd, :], in_=wq_s[kd])
                    nc.sync.dma_start(out=wv_sb[:, kd, :], in_=wv_s[kd])
            if c == 0:
                # chunk 0 is the PE-starved prologue: normalize each tile as
                # soon as its own stats land so the transposes start early
                for j, gt in enumerate(range(4)):
                    ln_tile(x_ts[j], mv1[:, gt, :])
                    _newton_rstd(nc, stats, mv1[:, gt, 1:2],
                                 rstd1[:, gt:gt + 1], 1)
                    h_bf = work.tile([128, D], BF, tag="h_bf", name="h_bf",
                                     bufs=2)
                    nc.vector.tensor_scalar(
                        out=h_bf, in0=x_ts[j], scalar1=mv1[:, gt, 0:1],
                        scalar2=rstd1[:, gt:gt + 1],
                        op0=ALU.subtract, op1=ALU.mult)
                    transpose6(h_bf, hT[:, :, ds(128 * gt, 128)], gt % 2 == 0)
            else:
                for j, gt in enumerate(range(4 * c, 4 * c + 4)):
                    ln_tile(x_ts[j], mv1[:, gt, :])
                _newton_rstd(nc, stats, mv1[:, ds(4 * c, 4), 1],
                             rstd1[:, ds(4 * c, 4)], 4)
                for j, gt in enumerate(range(4 * c, 4 * c + 4)):
                    h_bf = work.tile([128, D], BF, tag="h_bf", name="h_bf",
                                     bufs=2)
                    nc.vector.tensor_scalar(
                        out=h_bf, in0=x_ts[j], scalar1=mv1[:, gt, 0:1],
                        scalar2=rstd1[:, gt:gt + 1],
                        op0=ALU.subtract, op1=ALU.mult)
                    transpose6(h_bf, hT[:, :, ds(128 * gt, 128)], gt % 2 == 0)

            # ---- QKV chunk c ----
            csl = ds(512 * c, 512)
            for w_sb, t_sb, b_sb in ((wk_sb, KT, bk_sb), (wq_sb, QT, bq_sb)):
                ps = pmm.tile([128, 512], F32, tag="mm", name="ps_qk")
                for kd in range(KD):
                    nc.tensor.matmul(
                        ps, lhsT=w_sb[:, kd, 0:128], rhs=hT[:, kd, csl],
                        start=(kd == 0), stop=(kd == KD - 1),
                    )
                nc.vector.tensor_scalar(
                    out=t_sb[:, 0, csl], in0=ps, scalar1=b_sb[:, 0:1],
                    scalar2=None, op0=ALU.add)
            # head-2 Q and K col-tiled into concurrent halves of the PE
            ps_c = pmm.tile([128, 512], F32, tag="mm", name="ps_c")
            ps_d = pmm.tile([128, 512], F32, tag="mm", name="ps_d")
            for kd in range(KD):
                nc.tensor.matmul(
                    ps_c[0:64, :], lhsT=wq_sb[:, kd, 128:192],
                    rhs=hT[:, kd, csl],
                    start=(kd == 0), stop=(kd == KD - 1),
                    tile_position=(0, 0),
                )
                nc.tensor.matmul(
                    ps_d[64:128, :], lhsT=wk_sb[:, kd, 128:192],
                    rhs=hT[:, kd, csl],
                    start=(kd == 0), stop=(kd == KD - 1),
                    tile_position=(0, 64),
                )
            nc.vector.tensor_scalar(
                out=QT[0:64, 1, csl], in0=ps_c[0:64, :],
                scalar1=bq_sb[0:64, 1:2], scalar2=None, op0=ALU.add)
            nc.vector.tensor_scalar(
                out=KT[0:64, 1, csl], in0=ps_d[64:128, :],
                scalar1=bk_sb[0:64, 1:2], scalar2=None, op0=ALU.add)
            for gt in range(4 * c, 4 * c + 4):
                ps = pmm.tile([128, 512], F32, tag="mm", name="ps_v")
                for kd in range(KD):
                    nc.tensor.matmul(
                        ps[:, 0:DLOC],
                        lhsT=hT[:, kd, ds(128 * gt, 128)],
                        rhs=wv_sb[:, kd, :],
                        start=(kd == 0), stop=(kd == KD - 1),
                    )
                nc.vector.tensor_tensor(
                    out=Vx[:, gt, :, 0:64],
                    in0=ps[:, 0:DLOC].rearrange("p (h c) -> p h c", c=64),
                    in1=bv_rep[:, :].rearrange("p (h c) -> p h c", c=64),
                    op=ALU.add,
                )
            if c == 0:
                for ko in range(2):
                    nc.sync.dma_start(out=wo_sb[:, ko, :], in_=wo_s[ko])
            if c == 1:
                for kd in range(KD):
                    nc.sync.dma_start(out=w1_sb[:, kd, :], in_=w1_e[kd])
                for kh in range(KH):
                    nc.sync.dma_start(out=w2_sb[:, kh, :], in_=w2_e[kh])

            # ---- attention block c (q-range [512c, 512c+512)) ----
            OT = outw.tile([128, 2, 512], BF, tag="OT", name="OT")
            # rows 64-127 of the h2 lhsT tile multiply zero-padded Wo rows,
            # but must be finite
            nc.gpsimd.memset(OT[64:128, 1, :], 0.0)
            nkt = 4 * c + 4

            def qslice(kt):
                j = kt - 4 * c
                qo = 128 * j if j > 0 else 0
                return qo, 512 - qo, j >= 0

            def divide_evac(i, o_ps):
                # divide by the ones-column denominator: bf16-cast the denom
                # row on the scalar engine, rank-1 PE matmul broadcasts it to
                # 64 partitions, fast 64-lane reciprocal, multiply.  Head 1
                # lands partition-shifted into rows 64-127 of the merged OT.
                dn1 = stats.tile([1, 512], BF, tag="dn1", name="dn1", bufs=2)
                nc.scalar.copy(out=dn1, in_=o_ps[64:65, :])
                dnb = pmm.tile([64, 512], F32, tag="mm", name="dnb")
                nc.tensor.matmul(dnb, lhsT=ones_r, rhs=dn1,
                                 start=True, stop=True)
                rcb_sb = stats.tile([64, 512], F32, tag="rcb_sb",
                                    name="rcb_sb", bufs=2)
                nc.vector.reciprocal_approx_fast(out=rcb_sb, in_=dnb)
                dst = (OT[0:64, 0, :], OT[64:128, 0, :], OT[0:64, 1, :])[i]
                nc.vector.tensor_tensor(
                    out=dst, in0=o_ps[0:64, :], in1=rcb_sb, op=ALU.mult)

            # heads 0/1 paired: concurrent score matmuls in disjoint PE
            # row-groups + one two-head exp ACT; one-deep software pipeline
            # (scores kt+1 issued before att@V kt).
            o_ps0 = po.tile([65, 512], F32, tag="po", name="o_ps0")
            o_ps1 = po.tile([65, 512], F32, tag="po", name="o_ps1")

            def scores01(kt):
                qo, w, diag = qslice(kt)
                ps2 = psc.tile([128, 2, 512], F32, tag="sc", name="ps2")
                for i in (0, 1):
                    nc.tensor.matmul(
                        ps2[:, i, 0:w],
                        lhsT=head_qk(KT, i, 128 * kt, 128),
                        rhs=head_qk(QT, i, 512 * c + qo, w),
                        start=True, stop=True,
                    )
                ex2 = exps.tile([128, 2, 512], BF, tag="ex2", name="ex2")
                nc.scalar.activation(
                    out=ex2[:, :, 0:w], in_=ps2[:, 0:2, 0:w],
                    func=AF.Exp, scale=SCALE)
                if diag:
                    for i in (0, 1):
                        nc.vector.tensor_tensor(
                            out=ex2[:, i, 0:128], in0=ex2[:, i, 0:128],
                            in1=cmask, op=ALU.mult)
                return ex2

            def av01(kt, ex2):
                qo, w, _ = qslice(kt)
                for i, o_ps in ((0, o_ps0), (1, o_ps1)):
                    nc.tensor.matmul(
                        o_ps[:, qo:512],
                        lhsT=Vx[:, kt, i, :],
                        rhs=ex2[:, i, 0:w],
                        start=(kt == 0), stop=(kt == nkt - 1),
                    )

            pend = scores01(0)
            for kt in range(nkt):
                nxt = scores01(kt + 1) if kt + 1 < nkt else None
                av01(kt, pend)
                pend = nxt
            divide_evac(0, o_ps0)
            divide_evac(1, o_ps1)

            # MLP for the previous chunk's shard tile lands here: its
            # ReduceScatter has had a head-pair loop's worth of time, and its
            # PE work fills the head-2 exp bubbles
            if c >= 1:
                mlp_tile(c - 1)

            # head 2 (kd-tile 1, rows 0-63)
            o_ps2 = po.tile([65, 512], F32, tag="po", name="o_ps2")

            def scores2(kt):
                qo, w, diag = qslice(kt)
                ps2 = psc.tile([128, 2, 512], F32, tag="sc", name="ps2h")
                nc.tensor.matmul(
                    ps2[:, 0, 0:w],
                    lhsT=head_qk(KT, 2, 128 * kt, 128),
                    rhs=head_qk(QT, 2, 512 * c + qo, w),
                    start=True, stop=True,
                )
                ex1 = exps.tile([128, 512], BF, tag="ex1", name="ex1")
                nc.scalar.activation(
                    out=ex1[:, 0:w], in_=ps2[:, 0, 0:w],
                    func=AF.Exp, scale=SCALE)
                if diag:
                    nc.vector.tensor_tensor(
                        out=ex1[:, 0:128], in0=ex1[:, 0:128],
                        in1=cmask, op=ALU.mult)
                return ex1

            pend = scores2(0)
            for kt in range(nkt):
                nxt = scores2(kt + 1) if kt + 1 < nkt else None
                qo, w, _ = qslice(kt)
                nc.tensor.matmul(
                    o_ps2[:, qo:512],
                    lhsT=Vx[:, kt, 2, :],
                    rhs=pend[:, 0:w],
                    start=(kt == 0), stop=(kt == nkt - 1),
                )
                pend = nxt
            divide_evac(2, o_ps2)

            # ---- out-projection + chunked ReduceScatter ----
            for tq in range(4):
                ybf = outw.tile([128, D], BF, tag="ybf", name="ybf")
                for nsl, nsz in ((0, 512), (512, 256)):
                    ps_y = pmm.tile([128, 512], F32, tag="mm", name="ps_y")
                    for ko in range(2):
                        nc.tensor.matmul(
                            ps_y[:, 0:nsz],
                            lhsT=OT[:, ko, ds(128 * tq, 128)],
                            rhs=wo_sb[:, ko, ds(nsl, nsz)],
                            start=(ko == 0), stop=(ko == 1),
                        )
                    nc.vector.tensor_copy(out=ybf[:, ds(nsl, nsz)],
                                          in_=ps_y[:, 0:nsz])
                nc.sync.dma_start(out=cc_in[4 * c + tq], in_=ybf)
            nc.gpsimd.collective_compute(
                "ReduceScatter",
                ALU.add,
                replica_groups=[[0, 1, 2, 3], [4, 5, 6, 7]],
                ins=[cc_in[ds(4 * c, 4)]],
                outs=[cc_rs[ds(c, 1)]],
            )
        mlp_tile(NB - 1)

    return nc


def make_in_maps(x, Wq, Wk, Wv, Wo, W1, W2, g1, b1, g2, b2):
    """Host-side sharding: per-core input dicts (one NEFF, per-core data)."""
    x = np.ascontiguousarray(np.asarray(x, np.float32))
    g1 = np.asarray(g1, np.float32)
    b1 = np.asarray(b1, np.float32)
    g2 = np.asarray(g2, np.float32)
    b2 = np.asarray(b2, np.float32)
    Wq, Wk, Wv, Wo = (np.asarray(w, np.float32) for w in (Wq, Wk, Wv, Wo))
    W1, W2 = np.asarray(W1, np.float32), np.asarray(W2, np.float32)

    # fold LN gains into the weights; LN biases become per-output biases
    wq_g = g1[:, None] * Wq
    wk_g = g1[:, None] * Wk
    wv_g = g1[:, None] * Wv
    w1_g = g2[:, None] * W1
    bias_q = b1 @ Wq
    bias_k = b1 @ Wk
    bias_v = b1 @ Wv
    bias_1 = b2 @ W1

    w2_bf = W2.astype(BF_NP).reshape(KH, 128, D)
    w1_bf = w1_g.astype(BF_NP).reshape(KD, 128, DH)

    def pad_to(a, n):
        out = np.zeros((n,) + a.shape[1:], a.dtype)
        out[: a.shape[0]] = a
        return out

    in_maps = []
    for c in range(NCORES):
        b, r = divmod(c, GRP)
        hsl = slice(DLOC * r, DLOC * (r + 1))
        in_maps.append({
            "x_full": x[b].reshape(NT, 128, D).astype(BF_NP),
            "x_own": np.stack([x[b, 512 * k + 128 * r: 512 * k + 128 * (r + 1)]
                               for k in range(NSH)]),
            "wq_s": np.ascontiguousarray(wq_g[:, hsl]).astype(BF_NP)
                      .reshape(KD, 128, DLOC),
            "wk_s": np.ascontiguousarray(wk_g[:, hsl]).astype(BF_NP)
                      .reshape(KD, 128, DLOC),
            "wv_s": np.ascontiguousarray(wv_g[:, hsl]).astype(BF_NP)
                      .reshape(KD, 128, DLOC),
            "wo_s": pad_to(np.ascontiguousarray(Wo[hsl]), DPAD)
                      .astype(BF_NP).reshape(2, 128, D),
            "w1_e": w1_bf,
            "w2_e": w2_bf,
            "bq_s": pad_to(np.ascontiguousarray(bias_q[hsl]), DPAD)
                      .reshape(2, 128),
            "bk_s": pad_to(np.ascontiguousarray(bias_k[hsl]), DPAD)
                      .reshape(2, 128),
            "bv_s": np.ascontiguousarray(bias_v[hsl]),
            "b1_s": bias_1.astype(BF_NP),
        })
    return in_maps


def assemble_output(results):
    out = np.empty((B, T, D), np.float32)
    for core in range(NCORES):
        b, r = divmod(core, GRP)
        for c in range(NSH):
            out[b, 512 * c + 128 * r: 512 * c + 128 * (r + 1)] = \
                results[core]["y_out"][c]
    return out


_NC_CACHE = {}


def get_nc(mlp_bias=True):
    if mlp_bias not in _NC_CACHE:
        _NC_CACHE[mlp_bias] = build_nc(mlp_bias)
    return _NC_CACHE[mlp_bias]


def run(in_maps, **kwargs):
    mlp_bias = bool(np.any(np.asarray(in_maps[0]["b1_s"], np.float32)))
    nc = get_nc(mlp_bias)
    if not nc.is_finalized():
        nc.finalize()
    return run_bass_kernel_spmd(nc, in_maps, list(range(NCORES)), **kwargs)


def kernel(**inputs):
    in_maps = make_in_maps(**inputs)
    res = run(in_maps)
    return assemble_output(res.results)


if __name__ == "__main__":
    nc = build_nc()
    print("built OK")
